# revision 3
# baseline (speedup 1.0000x reference)
"""Multi-head attention (dense transformer block) on 8 Trainium2 NeuronCores.

Reference computation (per batch element b of 8):
    qkv = x @ w_qkv.T + b_qkv                  # [1024, 2304]
    q, k, v = split heads (12 heads, d=64)
    attn = softmax(q k^T / sqrt(d))
    out  = (attn v) reshaped @ w_proj.T + b_proj

Sharding: pure data parallel — core b handles batch element b, weights are
replicated, no collectives.

Per-core kernel phases (all matmuls in float32r = full PE rate):
  B: qk^T = [Wq*scale; Wk] x^T        -> [1536, 1024]  (features on partitions)
  C: v    = x Wv^T + b_v               -> [1024, 12*(64+1)]  (ones col per head)
  D: per head: scores^T = k^T q -> exp -> PV accumulate (ones col gives the
     softmax row-sums for free), normalize by reciprocal(rowsum)
  E: out = score w_proj^T + b_proj     (bias via ones-row matmul)
"""

import os
import sys

for _p in ("/opt/trn_rl_repo", "/root/.axon_site/_ro/trn_rl_repo"):
    if os.path.isdir(_p) and _p not in sys.path:
        sys.path.insert(0, _p)

import numpy as np

import concourse.bass as bass
import concourse.mybir as mybir
import concourse.tile as tile
from concourse import bacc
from concourse.bass_utils import run_bass_kernel_spmd

DIM = 768
N_HEAD = 12
HEAD_DIM = 64
SCALE = HEAD_DIM ** (-0.5)
NB = 8          # batch == number of cores
N = 1024        # sequence length
NQT = 2         # 1024 = 2 x 512 moving-dim tiles
CCH = DIM // 128  # 6 contraction chunks

F32 = mybir.dt.float32
F32R = mybir.dt.float32r
AF = mybir.ActivationFunctionType

_CACHE: dict = {}


def _build():
    nc = bacc.Bacc("TRN2", target_bir_lowering=False, debug=False)

    xT_d = nc.dram_tensor("xT", [DIM, N], F32R, kind="ExternalInput")
    wqk_d = nc.dram_tensor("w_qk_T", [DIM, 2 * DIM], F32R, kind="ExternalInput")
    bqk_d = nc.dram_tensor("b_qk_t", [128, 12], F32, kind="ExternalInput")
    wv_d = nc.dram_tensor("w_v_T", [DIM, DIM], F32R, kind="ExternalInput")
    bv_d = nc.dram_tensor("b_v", [1, DIM], F32R, kind="ExternalInput")
    wp_d = nc.dram_tensor("w_p_T", [DIM, DIM], F32R, kind="ExternalInput")
    bp_d = nc.dram_tensor("b_p", [1, DIM], F32R, kind="ExternalInput")
    out_d = nc.dram_tensor("out", [N, DIM], F32, kind="ExternalOutput")

    with tile.TileContext(nc) as tc:
        with (
            tc.tile_pool(name="consts", bufs=1) as consts,
            tc.tile_pool(name="qk", bufs=1) as qk_pool,
            tc.tile_pool(name="score", bufs=1) as score_pool,
            tc.tile_pool(name="ps", bufs=4, space="PSUM") as ps_pool,
            tc.tile_pool(name="small", bufs=4) as small_pool,
        ):
            ONE_F32_BITS = 0x3F800000  # f32r memset is invalid ISA; write bits
            ones_sb = consts.tile([1, 128], F32R)
            nc.vector.memset(ones_sb[:].bitcast(mybir.dt.uint32), ONE_F32_BITS)
            bqk_sb = consts.tile([128, 12], F32)
            nc.sync.dma_start(bqk_sb[:], bqk_d[:])
            bv_sb = consts.tile([1, DIM], F32R)
            nc.sync.dma_start(bv_sb[:], bv_d[:])
            bp_sb = consts.tile([1, DIM], F32R)
            nc.sync.dma_start(bp_sb[:], bp_d[:])

            qk_sb = qk_pool.tile([128, 12, N], F32R)      # [o=1536, n] transposed
            score_sb = score_pool.tile([128, CCH, N], F32R)  # [c=768, n]

            with tc.tile_pool(name="v", bufs=1) as v_pool:
                v_sb = v_pool.tile([128, 8, N_HEAD * 65], F32R)  # [n, h*(64+1)]

                with (
                    tc.tile_pool(name="x", bufs=1) as x_pool,
                    tc.tile_pool(name="wqk", bufs=1) as wqk_pool,
                    tc.tile_pool(name="wv", bufs=1) as wv_pool,
                ):
                    x_sb = x_pool.tile([128, CCH, N], F32R)
                    wqk_sb = wqk_pool.tile([128, CCH, 2 * DIM], F32R)
                    wv_sb = wv_pool.tile([128, CCH, DIM], F32R)

                    xT_v = xT_d[:].rearrange("(c p) n -> c p n", p=128)
                    wqk_v = wqk_d[:].rearrange("(c p) o -> c p o", p=128)
                    for c in range(CCH):
                        nc.sync.dma_start(x_sb[:, c, :], xT_v[c])
                        nc.sync.dma_start(wqk_sb[:, c, :], wqk_v[c])
                    nc.sync.dma_start(
                        wv_sb[:], wv_d[:].rearrange("(c p) o -> p c o", p=128)
                    )

                    # ---- Phase B: qk^T projection [1536, 1024] ----
                    for ot in range(12):
                        for nq in (0, 512):
                            ps = ps_pool.tile([128, 512], F32)
                            for c in range(CCH):
                                nc.tensor.matmul(
                                    ps[:],
                                    wqk_sb[:, c, ot * 128:(ot + 1) * 128],
                                    x_sb[:, c, nq:nq + 512],
                                    start=(c == 0),
                                    stop=(c == CCH - 1),
                                )
                            nc.vector.tensor_scalar_add(
                                qk_sb[:, ot, nq:nq + 512], ps[:],
                                bqk_sb[:, ot:ot + 1],
                            )

                    # ---- Phase C: v projection, natural layout + ones cols ----
                    v_ones = v_sb[:].rearrange("p n (h d) -> p n h d", d=65)[:, :, :, 64:65]
                    nc.vector.memset(v_ones.bitcast(mybir.dt.uint32), ONE_F32_BITS)
                    for nt in range(8):
                        for o0, ow, off in ((0, 512, 0), (512, 256, 8 * 65)):
                            ps = ps_pool.tile([128, 512], F32)
                            nc.tensor.matmul(
                                ps[:, :ow], ones_sb[:], bv_sb[:, o0:o0 + ow],
                                start=True, stop=False,
                            )
                            for c in range(CCH):
                                nc.tensor.matmul(
                                    ps[:, :ow],
                                    x_sb[:, c, nt * 128:(nt + 1) * 128],
                                    wv_sb[:, c, o0:o0 + ow],
                                    start=False,
                                    stop=(c == CCH - 1),
                                )
                            nh = ow // 64
                            src = ps[:, :ow].rearrange("p (h d) -> p h d", d=64)
                            dst = v_sb[:, nt, off:off + nh * 65].rearrange(
                                "p (h d) -> p h d", d=65
                            )[:, :, 0:64]
                            nc.vector.tensor_copy(dst, src)

                # ---- Phase D: attention per head pair ----
                with (
                    tc.tile_pool(name="raw", bufs=2) as raw_pool,
                    tc.tile_pool(name="attn", bufs=3) as attn_pool,
                    tc.tile_pool(name="acc", bufs=4, space="PSUM") as acc_pool,
                ):
                    for hp in range(6):
                        strips = {}
                        for qi, nq in enumerate((0, 512)):
                            for half, p0 in ((0, 0), (1, 64)):
                                raw = raw_pool.tile([128, 8, 512], F32)
                                for nk in range(8):
                                    ps = ps_pool.tile([128, 512], F32)
                                    nc.tensor.matmul(
                                        ps[:],
                                        qk_sb[p0:p0 + 64, 6 + hp, nk * 128:(nk + 1) * 128],
                                        qk_sb[p0:p0 + 64, hp, nq:nq + 512],
                                        start=True, stop=True,
                                        tile_position=(p0, 0),
                                    )
                                    nc.vector.tensor_copy(raw[:, nk, :], ps[:])
                                attn = attn_pool.tile([128, 8, 512], F32R)
                                nc.scalar.activation(
                                    attn[:].rearrange("p a b -> p (a b)"),
                                    raw[:].rearrange("p a b -> p (a b)"),
                                    AF.Exp,
                                )
                                strips[(qi, half)] = attn
                        for qi, nq in enumerate((0, 512)):
                            for half, p0 in ((0, 0), (1, 64)):
                                h = 2 * hp + half
                                attn = strips[(qi, half)]
                                acc = acc_pool.tile([65, 512], F32)
                                for nk in range(8):
                                    nc.tensor.matmul(
                                        acc[:],
                                        v_sb[:, nk, h * 65:(h + 1) * 65],
                                        attn[:, nk, :],
                                        start=(nk == 0),
                                        stop=(nk == 7),
                                    )
                                rec = small_pool.tile([1, 512], F32, tag="rec")
                                nc.vector.reciprocal(rec[:], acc[64:65, :])
                                bc = small_pool.tile([64, 512], F32, tag="bc")
                                nc.gpsimd.partition_broadcast(bc[:], rec[:], channels=64)
                                nc.vector.tensor_mul(
                                    score_sb[p0:p0 + 64, hp, nq:nq + 512],
                                    acc[0:64, :],
                                    bc[:],
                                )

            # ---- Phase E: output projection ----
            with (
                tc.tile_pool(name="wp", bufs=1) as wp_pool,
                tc.tile_pool(name="ostage", bufs=3) as out_pool,
            ):
                wp_sb = wp_pool.tile([128, CCH, DIM], F32R)
                nc.sync.dma_start(
                    wp_sb[:], wp_d[:].rearrange("(c p) o -> p c o", p=128)
                )
                for nt in range(8):
                    stage = out_pool.tile([128, DIM], F32)
                    for o0, ow in ((0, 512), (512, 256)):
                        ps = ps_pool.tile([128, 512], F32)
                        nc.tensor.matmul(
                            ps[:, :ow], ones_sb[:], bp_sb[:, o0:o0 + ow],
                            start=True, stop=False,
                        )
                        for c in range(CCH):
                            nc.tensor.matmul(
                                ps[:, :ow],
                                score_sb[:, c, nt * 128:(nt + 1) * 128],
                                wp_sb[:, c, o0:o0 + ow],
                                start=False,
                                stop=(c == CCH - 1),
                            )
                        nc.scalar.copy(stage[:, o0:o0 + ow], ps[:, :ow])
                    nc.sync.dma_start(out_d[nt * 128:(nt + 1) * 128, :], stage[:])

    nc.compile()
    return nc


def _get_nc():
    if "nc" not in _CACHE:
        _CACHE["nc"] = _build()
    return _CACHE["nc"]


def kernel(x, w_qkv, b_qkv, w_proj, b_proj, **run_kwargs):
    x = np.asarray(x, dtype=np.float32)
    w_qkv = np.asarray(w_qkv, dtype=np.float32)
    b_qkv = np.asarray(b_qkv, dtype=np.float32)
    w_proj = np.asarray(w_proj, dtype=np.float32)
    b_proj = np.asarray(b_proj, dtype=np.float32)

    # Host-side layout prep (no arithmetic beyond folding the 1/sqrt(d) scale
    # into the q projection).
    w_qk = w_qkv[: 2 * DIM].copy()
    b_qk = b_qkv[: 2 * DIM].copy()
    w_qk[:DIM] *= SCALE
    b_qk[:DIM] *= SCALE
    w_qk_T = np.ascontiguousarray(w_qk.T)                 # [768, 1536]
    b_qk_t = np.ascontiguousarray(b_qk.reshape(12, 128).T)  # [128, 12]
    w_v_T = np.ascontiguousarray(w_qkv[2 * DIM:].T)       # [768, 768]
    b_v = b_qkv[2 * DIM:].reshape(1, DIM).copy()
    w_p_T = np.ascontiguousarray(w_proj.T)                # [768, 768]
    b_p = b_proj.reshape(1, DIM).copy()

    nc = _get_nc()
    in_maps = []
    for b in range(NB):
        in_maps.append({
            "xT": np.ascontiguousarray(x[b].T),
            "w_qk_T": w_qk_T,
            "b_qk_t": b_qk_t,
            "w_v_T": w_v_T,
            "b_v": b_v,
            "w_p_T": w_p_T,
            "b_p": b_p,
        })
    res = run_bass_kernel_spmd(nc, in_maps, core_ids=list(range(NB)), **run_kwargs)
    out = np.stack([res.results[b]["out"] for b in range(NB)], axis=0)
    if run_kwargs:
        return out, res
    return out


if __name__ == "__main__":
    rng = np.random.default_rng(0)
    x = rng.standard_normal((NB, N, DIM), dtype=np.float32)
    w_qkv = rng.standard_normal((3 * DIM, DIM), dtype=np.float32) * DIM ** -0.5
    b_qkv = rng.standard_normal((3 * DIM,), dtype=np.float32) * 0.02
    w_proj = rng.standard_normal((DIM, DIM), dtype=np.float32) * DIM ** -0.5
    b_proj = rng.standard_normal((DIM,), dtype=np.float32) * 0.02
    out = kernel(x=x, w_qkv=w_qkv, b_qkv=b_qkv, w_proj=w_proj, b_proj=b_proj)
    print("out", out.shape, out.dtype, float(np.abs(out).mean()))


# revision 10
# speedup vs baseline: 1.1635x; 1.1635x over previous
"""Multi-head attention (dense transformer block) on 8 Trainium2 NeuronCores.

Reference computation (per batch element b of 8):
    qkv = x @ w_qkv.T + b_qkv                  # [1024, 2304]
    q, k, v = split heads (12 heads, d=64)
    attn = softmax(q k^T / sqrt(d))
    out  = (attn v) reshaped @ w_proj.T + b_proj

Sharding: pure data parallel — core b handles batch element b, weights are
replicated, no collectives.

Per-core kernel phases (all matmuls in float32r = full PE rate):
  B: qk^T = [Wq*scale; Wk] x^T        -> [1536, 1024]  (features on partitions)
  C: v    = x Wv^T + b_v               -> [1024, 12*(64+1)]  (ones col per head)
  D: per head: scores^T = k^T q -> exp -> PV accumulate (ones col gives the
     softmax row-sums for free), normalize by reciprocal(rowsum)
  E: out = score w_proj^T + b_proj     (bias via ones-row matmul)
"""

import os
import sys

for _p in ("/opt/trn_rl_repo", "/root/.axon_site/_ro/trn_rl_repo"):
    if os.path.isdir(_p) and _p not in sys.path:
        sys.path.insert(0, _p)

import numpy as np

import concourse.bass as bass
import concourse.mybir as mybir
import concourse.tile as tile
from concourse import bacc
from concourse.bass_utils import run_bass_kernel_spmd

DIM = 768
N_HEAD = 12
HEAD_DIM = 64
SCALE = HEAD_DIM ** (-0.5)
NB = 8          # batch == number of cores
N = 1024        # sequence length
NQT = 2         # 1024 = 2 x 512 moving-dim tiles
CCH = DIM // 128  # 6 contraction chunks

F32 = mybir.dt.float32
F32R = mybir.dt.float32r
AF = mybir.ActivationFunctionType

DEBUG = False

_CACHE: dict = {}


def _build():
    nc = bacc.Bacc("TRN2", target_bir_lowering=False, debug=False)

    xT_d = nc.dram_tensor("xT", [DIM, N], F32R, kind="ExternalInput")
    wqk_d = nc.dram_tensor("w_qk_T", [DIM, 2 * DIM], F32R, kind="ExternalInput")
    bqk_d = nc.dram_tensor("b_qk_t", [128, 12], F32, kind="ExternalInput")
    wv_d = nc.dram_tensor("w_v_T", [DIM, DIM], F32R, kind="ExternalInput")
    bv_d = nc.dram_tensor("b_v", [1, DIM], F32R, kind="ExternalInput")
    wp_d = nc.dram_tensor("w_p_T", [DIM, DIM], F32R, kind="ExternalInput")
    bp_d = nc.dram_tensor("b_p", [1, DIM], F32R, kind="ExternalInput")
    out_d = nc.dram_tensor("out", [N, DIM], F32, kind="ExternalOutput")
    if DEBUG:
        dbg_qk = nc.dram_tensor("dbg_qk", [12 * 128, N], F32, kind="ExternalOutput")
        dbg_v = nc.dram_tensor("dbg_v", [N, N_HEAD * 65], F32, kind="ExternalOutput")
        dbg_attn0 = nc.dram_tensor("dbg_attn0", [128, 1024], F32, kind="ExternalOutput")
        dbg_acc = nc.dram_tensor("dbg_acc", [65, 512], F32, kind="ExternalOutput")
        dbg_score = nc.dram_tensor("dbg_score", [CCH * 128, N], F32, kind="ExternalOutput")
        dbg_rec = nc.dram_tensor("dbg_rec", [1, 512], F32, kind="ExternalOutput")
        dbg_bc = nc.dram_tensor("dbg_bc", [64, 512], F32, kind="ExternalOutput")

    with tile.TileContext(nc) as tc:
        with (
            tc.tile_pool(name="consts", bufs=1) as consts,
            tc.tile_pool(name="qk", bufs=1) as qk_pool,
            tc.tile_pool(name="score", bufs=1) as score_pool,
            tc.tile_pool(name="ps", bufs=2, space="PSUM") as ps_pool,
            tc.tile_pool(name="small", bufs=4) as small_pool,
        ):
            ONE_F32_BITS = 0x3F800000  # f32r memset is invalid ISA; write bits
            ones_sb = consts.tile([1, 128], F32R)
            nc.vector.memset(ones_sb[:].bitcast(mybir.dt.uint32), ONE_F32_BITS)
            bqk_sb = consts.tile([128, 12], F32)
            nc.sync.dma_start(bqk_sb[:], bqk_d[:])
            bv_sb = consts.tile([1, DIM], F32R)
            nc.sync.dma_start(bv_sb[:], bv_d[:])
            bp_sb = consts.tile([1, DIM], F32R)
            nc.sync.dma_start(bp_sb[:], bp_d[:])

            qk_sb = qk_pool.tile([128, 12, N], F32R)      # [o=1536, n] transposed
            score_sb = score_pool.tile([128, CCH, N], F32R)  # [c=768, n]

            with tc.tile_pool(name="v", bufs=1) as v_pool:
                v_sb = v_pool.tile([128, 8, N_HEAD * 65], F32R)  # [n, h*(64+1)]

                with (
                    tc.tile_pool(name="x", bufs=1) as x_pool,
                    tc.tile_pool(name="wqk", bufs=1) as wqk_pool,
                    tc.tile_pool(name="wv", bufs=1) as wv_pool,
                ):
                    x_sb = x_pool.tile([128, CCH, N], F32R)
                    wqk_sb = wqk_pool.tile([128, CCH, 2 * DIM], F32R)
                    wv_sb = wv_pool.tile([128, CCH, DIM], F32R)

                    xT_v = xT_d[:].rearrange("(c p) n -> c p n", p=128)
                    wqk_v = wqk_d[:].rearrange("(c p) o -> c p o", p=128)
                    for c in range(CCH):
                        nc.sync.dma_start(x_sb[:, c, :], xT_v[c])
                        nc.sync.dma_start(wqk_sb[:, c, :], wqk_v[c])
                    nc.sync.dma_start(
                        wv_sb[:], wv_d[:].rearrange("(c p) o -> p c o", p=128)
                    )

                    # ---- Phase B: qk^T projection [1536, 1024] ----
                    for ot in range(12):
                        for nq in (0, 512):
                            ps = ps_pool.tile([128, 512], F32)
                            for c in range(CCH):
                                nc.tensor.matmul(
                                    ps[:],
                                    wqk_sb[:, c, ot * 128:(ot + 1) * 128],
                                    x_sb[:, c, nq:nq + 512],
                                    start=(c == 0),
                                    stop=(c == CCH - 1),
                                )
                            nc.vector.tensor_scalar_add(
                                qk_sb[:, ot, nq:nq + 512], ps[:],
                                bqk_sb[:, ot:ot + 1],
                            )

                    # ---- Phase C: v projection, natural layout + ones cols ----
                    v_ones = v_sb[:].rearrange("p n (h d) -> p n h d", d=65)[:, :, :, 64:65]
                    nc.vector.memset(v_ones.bitcast(mybir.dt.uint32), ONE_F32_BITS)
                    for nt in range(8):
                        for o0, ow, off in ((0, 512, 0), (512, 256, 8 * 65)):
                            ps = ps_pool.tile([128, 512], F32)
                            nc.tensor.matmul(
                                ps[:, :ow], ones_sb[:], bv_sb[:, o0:o0 + ow],
                                start=True, stop=False,
                            )
                            for c in range(CCH):
                                nc.tensor.matmul(
                                    ps[:, :ow],
                                    x_sb[:, c, nt * 128:(nt + 1) * 128],
                                    wv_sb[:, c, o0:o0 + ow],
                                    start=False,
                                    stop=(c == CCH - 1),
                                )
                            nh = ow // 64
                            src = ps[:, :ow].rearrange("p (h d) -> p h d", d=64)
                            dst = v_sb[:, nt, off:off + nh * 65].rearrange(
                                "p (h d) -> p h d", d=65
                            )[:, :, 0:64]
                            nc.vector.tensor_copy(dst, src)

                if DEBUG:
                    nc.sync.dma_start(
                        dbg_qk[:].rearrange("(t p) n -> p t n", p=128),
                        qk_sb[:].bitcast(F32))
                    nc.sync.dma_start(
                        dbg_v[:].rearrange("(t p) c -> p t c", p=128),
                        v_sb[:].bitcast(F32))

                # ---- Phases D + E under the wp pool so its DMA overlaps D ----
                with tc.tile_pool(name="wp", bufs=1) as wp_pool:
                    wp_sb = wp_pool.tile([128, CCH, DIM], F32R)
                    nc.sync.dma_start(
                        wp_sb[:], wp_d[:].rearrange("(c p) o -> p c o", p=128)
                    )

                    # ---- Phase D: attention per head pair ----
                    with (
                        tc.tile_pool(name="attn", bufs=12) as attn_pool,
                        tc.tile_pool(name="pair", bufs=2, space="PSUM") as pair_pool,
                        tc.tile_pool(name="acc", bufs=2, space="PSUM") as acc_pool,
                    ):
                        def emit_scores(hp, nq, p0):
                            """8 nk-tiles of scoresT for one head/nq, exp'd in
                            [128,1024] PSUM pairs straight to SBUF attn tiles."""
                            attns = []
                            for pk in range(4):
                                pair = pair_pool.tile([128, 1024], F32)
                                for j in range(2):
                                    nk = 2 * pk + j
                                    nc.tensor.matmul(
                                        pair[:, j * 512:(j + 1) * 512],
                                        qk_sb[p0:p0 + 64, 6 + hp, nk * 128:(nk + 1) * 128],
                                        qk_sb[p0:p0 + 64, hp, nq:nq + 512],
                                        start=True, stop=True,
                                        tile_position=(p0, 0),
                                    )
                                at = attn_pool.tile([128, 1024], F32R)
                                nc.scalar.activation(at[:], pair[:], AF.Exp)
                                if DEBUG and hp == 0 and nq == 0 and p0 == 0 and pk == 0:
                                    nc.sync.dma_start(dbg_attn0[:], at[:].bitcast(F32))
                                attns.append(at)
                            return attns

                        def emit_pv(hp, nq, p0, half, attns):
                            h = 2 * hp + half
                            acc = acc_pool.tile([65, 512], F32)
                            for nk in range(8):
                                nc.tensor.matmul(
                                    acc[:],
                                    v_sb[:, nk, h * 65:(h + 1) * 65],
                                    attns[nk // 2][:, (nk % 2) * 512:(nk % 2 + 1) * 512],
                                    start=(nk == 0),
                                    stop=(nk == 7),
                                )
                            if DEBUG and hp == 0 and nq == 0 and half == 0:
                                acc_cp = small_pool.tile([65, 512], F32, tag="dbgacc")
                                nc.vector.tensor_copy(acc_cp[:], acc[:])
                                nc.sync.dma_start(dbg_acc[:], acc_cp[:])
                            # custom-DVE ops mis-read PSUM APs at partition
                            # offsets > 0 — stage the rowsum to SBUF first.
                            rs = small_pool.tile([1, 512], F32, tag="rs")
                            nc.vector.tensor_copy(rs[:], acc[64:65, :])
                            rec = small_pool.tile([1, 512], F32, tag="rec")
                            nc.vector.reciprocal_approx_fast(rec[:], rs[:])
                            bc = small_pool.tile([64, 512], F32, tag="bc")
                            nc.gpsimd.partition_broadcast(bc[:], rec[:], channels=64)
                            if DEBUG and hp == 0 and nq == 0 and half == 0:
                                nc.sync.dma_start(dbg_rec[:], rec[:])
                            nc.vector.tensor_mul(
                                score_sb[p0:p0 + 64, hp, nq:nq + 512],
                                acc[0:64, :],
                                bc[:],
                            )

                        for hp in range(6):
                            # A0 B0 A1 | pvA0 | B1 | pvB0 pvA1 pvB1: keeps PE
                            # ~24 matmuls ahead of the exp each PV depends on.
                            a0 = emit_scores(hp, 0, 0)
                            b0 = emit_scores(hp, 0, 64)
                            a1 = emit_scores(hp, 512, 0)
                            emit_pv(hp, 0, 0, 0, a0)
                            b1 = emit_scores(hp, 512, 64)
                            emit_pv(hp, 0, 64, 1, b0)
                            emit_pv(hp, 512, 0, 0, a1)
                            emit_pv(hp, 512, 64, 1, b1)

                    if DEBUG:
                        nc.sync.dma_start(
                            dbg_score[:].rearrange("(t p) n -> p t n", p=128),
                            score_sb[:].bitcast(F32))

                    # ---- Phase E: output projection ----
                    with tc.tile_pool(name="ostage", bufs=3) as out_pool:
                        for nt in range(8):
                            stage = out_pool.tile([128, DIM], F32)
                            for o0, ow in ((0, 512), (512, 256)):
                                ps = ps_pool.tile([128, 512], F32)
                                nc.tensor.matmul(
                                    ps[:, :ow], ones_sb[:], bp_sb[:, o0:o0 + ow],
                                    start=True, stop=False,
                                )
                                for c in range(CCH):
                                    nc.tensor.matmul(
                                        ps[:, :ow],
                                        score_sb[:, c, nt * 128:(nt + 1) * 128],
                                        wp_sb[:, c, o0:o0 + ow],
                                        start=False,
                                        stop=(c == CCH - 1),
                                    )
                                nc.vector.tensor_copy(stage[:, o0:o0 + ow], ps[:, :ow])
                            nc.sync.dma_start(out_d[nt * 128:(nt + 1) * 128, :], stage[:])

    nc.compile()
    return nc


def _get_nc():
    if "nc" not in _CACHE:
        _CACHE["nc"] = _build()
    return _CACHE["nc"]


def kernel(x, w_qkv, b_qkv, w_proj, b_proj, **run_kwargs):
    x = np.asarray(x, dtype=np.float32)
    w_qkv = np.asarray(w_qkv, dtype=np.float32)
    b_qkv = np.asarray(b_qkv, dtype=np.float32)
    w_proj = np.asarray(w_proj, dtype=np.float32)
    b_proj = np.asarray(b_proj, dtype=np.float32)

    # Host-side layout prep (no arithmetic beyond folding the 1/sqrt(d) scale
    # into the q projection).
    w_qk = w_qkv[: 2 * DIM].copy()
    b_qk = b_qkv[: 2 * DIM].copy()
    w_qk[:DIM] *= SCALE
    b_qk[:DIM] *= SCALE
    w_qk_T = np.ascontiguousarray(w_qk.T)                 # [768, 1536]
    b_qk_t = np.ascontiguousarray(b_qk.reshape(12, 128).T)  # [128, 12]
    w_v_T = np.ascontiguousarray(w_qkv[2 * DIM:].T)       # [768, 768]
    b_v = b_qkv[2 * DIM:].reshape(1, DIM).copy()
    w_p_T = np.ascontiguousarray(w_proj.T)                # [768, 768]
    b_p = b_proj.reshape(1, DIM).copy()

    nc = _get_nc()
    in_maps = []
    for b in range(NB):
        in_maps.append({
            "xT": np.ascontiguousarray(x[b].T),
            "w_qk_T": w_qk_T,
            "b_qk_t": b_qk_t,
            "w_v_T": w_v_T,
            "b_v": b_v,
            "w_p_T": w_p_T,
            "b_p": b_p,
        })
    res = run_bass_kernel_spmd(nc, in_maps, core_ids=list(range(NB)), **run_kwargs)
    out = np.stack([res.results[b]["out"] for b in range(NB)], axis=0)
    if run_kwargs:
        return out, res
    return out


if __name__ == "__main__":
    rng = np.random.default_rng(0)
    x = rng.standard_normal((NB, N, DIM), dtype=np.float32)
    w_qkv = rng.standard_normal((3 * DIM, DIM), dtype=np.float32) * DIM ** -0.5
    b_qkv = rng.standard_normal((3 * DIM,), dtype=np.float32) * 0.02
    w_proj = rng.standard_normal((DIM, DIM), dtype=np.float32) * DIM ** -0.5
    b_proj = rng.standard_normal((DIM,), dtype=np.float32) * 0.02
    out = kernel(x=x, w_qkv=w_qkv, b_qkv=b_qkv, w_proj=w_proj, b_proj=b_proj)
    print("out", out.shape, out.dtype, float(np.abs(out).mean()))


# revision 11
# speedup vs baseline: 1.5310x; 1.3159x over previous
"""Multi-head attention (dense transformer block) on 8 Trainium2 NeuronCores.

Reference computation (per batch element b of 8):
    qkv = x @ w_qkv.T + b_qkv                  # [1024, 2304]
    q, k, v = split heads (12 heads, d=64)
    attn = softmax(q k^T / sqrt(d))
    out  = (attn v) reshaped @ w_proj.T + b_proj

Sharding: pure data parallel — core b handles batch element b, weights are
replicated, no collectives.

Per-core kernel phases (all matmuls in float32r = full PE rate):
  B: qk^T = [Wq*scale; Wk] x^T        -> [1536, 1024]  (features on partitions)
  C: v    = x Wv^T + b_v               -> [1024, 12*(64+1)]  (ones col per head)
  D: per head: scores^T = k^T q -> exp -> PV accumulate (ones col gives the
     softmax row-sums for free), normalize by reciprocal(rowsum)
  E: out = score w_proj^T + b_proj     (bias via ones-row matmul)
"""

import os
import sys

for _p in ("/opt/trn_rl_repo", "/root/.axon_site/_ro/trn_rl_repo"):
    if os.path.isdir(_p) and _p not in sys.path:
        sys.path.insert(0, _p)

import numpy as np

import concourse.bass as bass
import concourse.mybir as mybir
import concourse.tile as tile
from concourse import bacc
from concourse.bass_utils import run_bass_kernel_spmd

DIM = 768
N_HEAD = 12
HEAD_DIM = 64
SCALE = HEAD_DIM ** (-0.5)
NB = 8          # batch == number of cores
N = 1024        # sequence length
NQT = 2         # 1024 = 2 x 512 moving-dim tiles
CCH = DIM // 128  # 6 contraction chunks

F32 = mybir.dt.float32
F32R = mybir.dt.float32r
F16 = mybir.dt.float16
AF = mybir.ActivationFunctionType

DEBUG = False

_CACHE: dict = {}


def _build():
    nc = bacc.Bacc("TRN2", target_bir_lowering=False, debug=False)

    xT_d = nc.dram_tensor("xT", [DIM, N], F16, kind="ExternalInput")
    wqk_d = nc.dram_tensor("w_qk_T", [DIM, 2 * DIM], F16, kind="ExternalInput")
    bqk_d = nc.dram_tensor("b_qk_t", [128, 12], F32, kind="ExternalInput")
    wv_d = nc.dram_tensor("w_v_T", [DIM, DIM], F16, kind="ExternalInput")
    bv_d = nc.dram_tensor("b_v", [1, DIM], F16, kind="ExternalInput")
    wp_d = nc.dram_tensor("w_p_T", [DIM, DIM], F16, kind="ExternalInput")
    bp_d = nc.dram_tensor("b_p", [1, DIM], F16, kind="ExternalInput")
    out_d = nc.dram_tensor("out", [N, DIM], F32, kind="ExternalOutput")
    if DEBUG:
        dbg_qk = nc.dram_tensor("dbg_qk", [12 * 128, N], F16, kind="ExternalOutput")
        dbg_v = nc.dram_tensor("dbg_v", [N, N_HEAD * 65], F16, kind="ExternalOutput")
        dbg_attn0 = nc.dram_tensor("dbg_attn0", [128, 1024], F16, kind="ExternalOutput")
        dbg_acc = nc.dram_tensor("dbg_acc", [65, 512], F32, kind="ExternalOutput")
        dbg_score = nc.dram_tensor("dbg_score", [CCH * 128, N], F16, kind="ExternalOutput")
        dbg_rec = nc.dram_tensor("dbg_rec", [1, 512], F32, kind="ExternalOutput")
        dbg_bc = nc.dram_tensor("dbg_bc", [64, 512], F32, kind="ExternalOutput")

    with tile.TileContext(nc) as tc:
        with (
            tc.tile_pool(name="consts", bufs=1) as consts,
            tc.tile_pool(name="qk", bufs=1) as qk_pool,
            tc.tile_pool(name="score", bufs=1) as score_pool,
            tc.tile_pool(name="ps", bufs=2, space="PSUM") as ps_pool,
            tc.tile_pool(name="small", bufs=4) as small_pool,
        ):
            ones_sb = consts.tile([1, 128], F16)
            nc.vector.memset(ones_sb[:], 1.0)
            bqk_sb = consts.tile([128, 12], F32)
            nc.sync.dma_start(bqk_sb[:], bqk_d[:])
            bv_sb = consts.tile([1, DIM], F16)
            nc.sync.dma_start(bv_sb[:], bv_d[:])
            bp_sb = consts.tile([1, DIM], F16)
            nc.sync.dma_start(bp_sb[:], bp_d[:])

            qk_sb = qk_pool.tile([128, 12, N], F16)      # [o=1536, n] transposed
            score_sb = score_pool.tile([128, CCH, N], F16)  # [c=768, n]

            with tc.tile_pool(name="v", bufs=1) as v_pool:
                v_sb = v_pool.tile([128, 8, N_HEAD * 65], F16)  # [n, h*(64+1)]

                with (
                    tc.tile_pool(name="x", bufs=1) as x_pool,
                    tc.tile_pool(name="wqk", bufs=1) as wqk_pool,
                    tc.tile_pool(name="wv", bufs=1) as wv_pool,
                ):
                    x_sb = x_pool.tile([128, CCH, N], F16)
                    wqk_sb = wqk_pool.tile([128, CCH, 2 * DIM], F16)
                    wv_sb = wv_pool.tile([128, CCH, DIM], F16)

                    xT_v = xT_d[:].rearrange("(c p) n -> c p n", p=128)
                    wqk_v = wqk_d[:].rearrange("(c p) o -> c p o", p=128)
                    for c in range(CCH):
                        nc.sync.dma_start(x_sb[:, c, :], xT_v[c])
                        nc.sync.dma_start(wqk_sb[:, c, :], wqk_v[c])
                    nc.sync.dma_start(
                        wv_sb[:], wv_d[:].rearrange("(c p) o -> p c o", p=128)
                    )

                    # ---- Phase B: qk^T projection [1536, 1024] ----
                    for ot in range(12):
                        for nq in (0, 512):
                            ps = ps_pool.tile([128, 512], F32)
                            for c in range(CCH):
                                nc.tensor.matmul(
                                    ps[:],
                                    wqk_sb[:, c, ot * 128:(ot + 1) * 128],
                                    x_sb[:, c, nq:nq + 512],
                                    start=(c == 0),
                                    stop=(c == CCH - 1),
                                )
                            nc.vector.tensor_scalar_add(
                                qk_sb[:, ot, nq:nq + 512], ps[:],
                                bqk_sb[:, ot:ot + 1],
                            )

                    # ---- Phase C: v projection, natural layout + ones cols ----
                    v_ones = v_sb[:].rearrange("p n (h d) -> p n h d", d=65)[:, :, :, 64:65]
                    nc.vector.memset(v_ones, 1.0)
                    for nt in range(8):
                        for o0, ow, off in ((0, 512, 0), (512, 256, 8 * 65)):
                            ps = ps_pool.tile([128, 512], F32)
                            nc.tensor.matmul(
                                ps[:, :ow], ones_sb[:], bv_sb[:, o0:o0 + ow],
                                start=True, stop=False,
                            )
                            for c in range(CCH):
                                nc.tensor.matmul(
                                    ps[:, :ow],
                                    x_sb[:, c, nt * 128:(nt + 1) * 128],
                                    wv_sb[:, c, o0:o0 + ow],
                                    start=False,
                                    stop=(c == CCH - 1),
                                )
                            nh = ow // 64
                            src = ps[:, :ow].rearrange("p (h d) -> p h d", d=64)
                            dst = v_sb[:, nt, off:off + nh * 65].rearrange(
                                "p (h d) -> p h d", d=65
                            )[:, :, 0:64]
                            nc.vector.tensor_copy(dst, src)

                if DEBUG:
                    nc.sync.dma_start(
                        dbg_qk[:].rearrange("(t p) n -> p t n", p=128),
                        qk_sb[:])
                    nc.sync.dma_start(
                        dbg_v[:].rearrange("(t p) c -> p t c", p=128),
                        v_sb[:])

                # ---- Phases D + E under the wp pool so its DMA overlaps D ----
                with tc.tile_pool(name="wp", bufs=1) as wp_pool:
                    wp_sb = wp_pool.tile([128, CCH, DIM], F16)
                    nc.sync.dma_start(
                        wp_sb[:], wp_d[:].rearrange("(c p) o -> p c o", p=128)
                    )

                    # ---- Phase D: attention per head pair ----
                    with (
                        tc.tile_pool(name="attn", bufs=12) as attn_pool,
                        tc.tile_pool(name="pair", bufs=2, space="PSUM") as pair_pool,
                        tc.tile_pool(name="acc", bufs=2, space="PSUM") as acc_pool,
                    ):
                        def emit_scores(hp, nq, p0):
                            """8 nk-tiles of scoresT for one head/nq, exp'd in
                            [128,1024] PSUM pairs straight to SBUF attn tiles."""
                            attns = []
                            for pk in range(4):
                                pair = pair_pool.tile([128, 1024], F32)
                                for j in range(2):
                                    nk = 2 * pk + j
                                    nc.tensor.matmul(
                                        pair[:, j * 512:(j + 1) * 512],
                                        qk_sb[p0:p0 + 64, 6 + hp, nk * 128:(nk + 1) * 128],
                                        qk_sb[p0:p0 + 64, hp, nq:nq + 512],
                                        start=True, stop=True,
                                        tile_position=(p0, 0),
                                    )
                                at = attn_pool.tile([128, 1024], F16)
                                nc.scalar.activation(at[:], pair[:], AF.Exp)
                                if DEBUG and hp == 0 and nq == 0 and p0 == 0 and pk == 0:
                                    nc.sync.dma_start(dbg_attn0[:], at[:])
                                attns.append(at)
                            return attns

                        def emit_pv(hp, nq, p0, half, attns):
                            h = 2 * hp + half
                            acc = acc_pool.tile([65, 512], F32)
                            for nk in range(8):
                                nc.tensor.matmul(
                                    acc[:],
                                    v_sb[:, nk, h * 65:(h + 1) * 65],
                                    attns[nk // 2][:, (nk % 2) * 512:(nk % 2 + 1) * 512],
                                    start=(nk == 0),
                                    stop=(nk == 7),
                                )
                            if DEBUG and hp == 0 and nq == 0 and half == 0:
                                acc_cp = small_pool.tile([65, 512], F32, tag="dbgacc")
                                nc.vector.tensor_copy(acc_cp[:], acc[:])
                                nc.sync.dma_start(dbg_acc[:], acc_cp[:])
                            # custom-DVE ops mis-read PSUM APs at partition
                            # offsets > 0 — stage the rowsum to SBUF first.
                            rs = small_pool.tile([1, 512], F32, tag="rs")
                            nc.vector.tensor_copy(rs[:], acc[64:65, :])
                            rec = small_pool.tile([1, 512], F32, tag="rec")
                            nc.vector.reciprocal_approx_fast(rec[:], rs[:])
                            bc = small_pool.tile([64, 512], F32, tag="bc")
                            nc.gpsimd.partition_broadcast(bc[:], rec[:], channels=64)
                            if DEBUG and hp == 0 and nq == 0 and half == 0:
                                nc.sync.dma_start(dbg_rec[:], rec[:])
                            nc.vector.tensor_mul(
                                score_sb[p0:p0 + 64, hp, nq:nq + 512],
                                acc[0:64, :],
                                bc[:],
                            )

                        for hp in range(6):
                            # A0 B0 A1 | pvA0 | B1 | pvB0 pvA1 pvB1: keeps PE
                            # ~24 matmuls ahead of the exp each PV depends on.
                            a0 = emit_scores(hp, 0, 0)
                            b0 = emit_scores(hp, 0, 64)
                            a1 = emit_scores(hp, 512, 0)
                            emit_pv(hp, 0, 0, 0, a0)
                            b1 = emit_scores(hp, 512, 64)
                            emit_pv(hp, 0, 64, 1, b0)
                            emit_pv(hp, 512, 0, 0, a1)
                            emit_pv(hp, 512, 64, 1, b1)

                    if DEBUG:
                        nc.sync.dma_start(
                            dbg_score[:].rearrange("(t p) n -> p t n", p=128),
                            score_sb[:])

                    # ---- Phase E: output projection ----
                    with tc.tile_pool(name="ostage", bufs=3) as out_pool:
                        for nt in range(8):
                            stage = out_pool.tile([128, DIM], F32)
                            for o0, ow in ((0, 512), (512, 256)):
                                ps = ps_pool.tile([128, 512], F32)
                                nc.tensor.matmul(
                                    ps[:, :ow], ones_sb[:], bp_sb[:, o0:o0 + ow],
                                    start=True, stop=False,
                                )
                                for c in range(CCH):
                                    nc.tensor.matmul(
                                        ps[:, :ow],
                                        score_sb[:, c, nt * 128:(nt + 1) * 128],
                                        wp_sb[:, c, o0:o0 + ow],
                                        start=False,
                                        stop=(c == CCH - 1),
                                    )
                                nc.vector.tensor_copy(stage[:, o0:o0 + ow], ps[:, :ow])
                            nc.sync.dma_start(out_d[nt * 128:(nt + 1) * 128, :], stage[:])

    nc.compile()
    return nc


def _get_nc():
    if "nc" not in _CACHE:
        _CACHE["nc"] = _build()
    return _CACHE["nc"]


def kernel(x, w_qkv, b_qkv, w_proj, b_proj, **run_kwargs):
    x = np.asarray(x, dtype=np.float32)
    w_qkv = np.asarray(w_qkv, dtype=np.float32)
    b_qkv = np.asarray(b_qkv, dtype=np.float32)
    w_proj = np.asarray(w_proj, dtype=np.float32)
    b_proj = np.asarray(b_proj, dtype=np.float32)

    # Host-side layout prep (no arithmetic beyond folding the 1/sqrt(d) scale
    # into the q projection).
    w_qk = w_qkv[: 2 * DIM].copy()
    b_qk = b_qkv[: 2 * DIM].copy()
    w_qk[:DIM] *= SCALE
    b_qk[:DIM] *= SCALE
    w_qk_T = np.ascontiguousarray(w_qk.T)                 # [768, 1536]
    b_qk_t = np.ascontiguousarray(b_qk.reshape(12, 128).T)  # [128, 12]
    w_v_T = np.ascontiguousarray(w_qkv[2 * DIM:].T)       # [768, 768]
    b_v = b_qkv[2 * DIM:].reshape(1, DIM).copy()
    w_p_T = np.ascontiguousarray(w_proj.T)                # [768, 768]
    b_p = b_proj.reshape(1, DIM).copy()

    nc = _get_nc()
    w_qk_T16 = w_qk_T.astype(np.float16)
    w_v_T16 = w_v_T.astype(np.float16)
    w_p_T16 = w_p_T.astype(np.float16)
    b_v16 = b_v.astype(np.float16)
    b_p16 = b_p.astype(np.float16)
    in_maps = []
    for b in range(NB):
        in_maps.append({
            "xT": np.ascontiguousarray(x[b].T).astype(np.float16),
            "w_qk_T": w_qk_T16,
            "b_qk_t": b_qk_t,
            "w_v_T": w_v_T16,
            "b_v": b_v16,
            "w_p_T": w_p_T16,
            "b_p": b_p16,
        })
    res = run_bass_kernel_spmd(nc, in_maps, core_ids=list(range(NB)), **run_kwargs)
    out = np.stack([res.results[b]["out"] for b in range(NB)], axis=0)
    if run_kwargs:
        return out, res
    return out


if __name__ == "__main__":
    rng = np.random.default_rng(0)
    x = rng.standard_normal((NB, N, DIM), dtype=np.float32)
    w_qkv = rng.standard_normal((3 * DIM, DIM), dtype=np.float32) * DIM ** -0.5
    b_qkv = rng.standard_normal((3 * DIM,), dtype=np.float32) * 0.02
    w_proj = rng.standard_normal((DIM, DIM), dtype=np.float32) * DIM ** -0.5
    b_proj = rng.standard_normal((DIM,), dtype=np.float32) * 0.02
    out = kernel(x=x, w_qkv=w_qkv, b_qkv=b_qkv, w_proj=w_proj, b_proj=b_proj)
    print("out", out.shape, out.dtype, float(np.abs(out).mean()))


# revision 12
# speedup vs baseline: 1.8973x; 1.2392x over previous
"""Multi-head attention (dense transformer block) on 8 Trainium2 NeuronCores.

Reference computation (per batch element b of 8):
    qkv = x @ w_qkv.T + b_qkv                  # [1024, 2304]
    q, k, v = split heads (12 heads, d=64)
    attn = softmax(q k^T / sqrt(d))
    out  = (attn v) reshaped @ w_proj.T + b_proj

Sharding: pure data parallel — core b handles batch element b, weights are
replicated, no collectives.

Per-core kernel (all matmul operands fp16, fp32 PSUM accumulation):
  C: v    = x Wv^T + b_v            -> [1024, 12*(64+1)] (ones col per head
     makes the PV matmul emit softmax row-sums for free)
  B: qk^T = [Wq*scale; Wk] x^T      -> [1536, 1024] (features on partitions)
  D: per head pair hp: scores^T = k^T q (two heads row-tiled into the two
     halves of the PE array), exp on ScalarE straight from PSUM pairs,
     PV accumulate, normalize by approx-reciprocal(rowsum).
     B tiles for wave hp+1 and PV of wave hp-1 are interleaved between the
     score matmuls so the PE never idles while ScalarE runs the exps.
  E: out = score w_proj^T + b_proj  (bias via ones-row matmul)
"""

import os
import sys

for _p in ("/opt/trn_rl_repo", "/root/.axon_site/_ro/trn_rl_repo"):
    if os.path.isdir(_p) and _p not in sys.path:
        sys.path.insert(0, _p)

import numpy as np

import concourse.bass as bass
import concourse.mybir as mybir
import concourse.tile as tile
from concourse import bacc
from concourse.bass_utils import run_bass_kernel_spmd

DIM = 768
N_HEAD = 12
HEAD_DIM = 64
SCALE = HEAD_DIM ** (-0.5)
NB = 8          # batch == number of cores
N = 1024        # sequence length
CCH = DIM // 128  # 6 contraction chunks

F32 = mybir.dt.float32
F16 = mybir.dt.float16
AF = mybir.ActivationFunctionType

_CACHE: dict = {}


def _build():
    nc = bacc.Bacc("TRN2", target_bir_lowering=False, debug=False)

    xT_d = nc.dram_tensor("xT", [DIM, N], F16, kind="ExternalInput")
    wqk_d = nc.dram_tensor("w_qk_T", [DIM, 2 * DIM], F16, kind="ExternalInput")
    bqk_d = nc.dram_tensor("b_qk_t", [128, 12], F32, kind="ExternalInput")
    wv_d = nc.dram_tensor("w_v_T", [DIM, DIM], F16, kind="ExternalInput")
    bv_d = nc.dram_tensor("b_v", [1, DIM], F16, kind="ExternalInput")
    wp_d = nc.dram_tensor("w_p_T", [DIM, DIM], F16, kind="ExternalInput")
    bp_d = nc.dram_tensor("b_p", [1, DIM], F16, kind="ExternalInput")
    out_d = nc.dram_tensor("out", [N, DIM], F32, kind="ExternalOutput")

    with tile.TileContext(nc) as tc:
        with (
            tc.tile_pool(name="consts", bufs=1) as consts,
            tc.tile_pool(name="qk", bufs=1) as qk_pool,
            tc.tile_pool(name="score", bufs=1) as score_pool,
            tc.tile_pool(name="v", bufs=1) as v_pool,
            tc.tile_pool(name="x", bufs=1) as x_pool,
            tc.tile_pool(name="wqk", bufs=1) as wqk_pool,
            tc.tile_pool(name="wv", bufs=1) as wv_pool,
            tc.tile_pool(name="wp", bufs=1) as wp_pool,
            tc.tile_pool(name="attn", bufs=32) as attn_pool,
            tc.tile_pool(name="small", bufs=4) as small_pool,
            tc.tile_pool(name="ostage", bufs=3) as out_pool,
            tc.tile_pool(name="ps", bufs=2, space="PSUM") as ps_pool,
            tc.tile_pool(name="pair", bufs=2, space="PSUM") as pair_pool,
            tc.tile_pool(name="acc", bufs=2, space="PSUM") as acc_pool,
        ):
            x_sb = x_pool.tile([128, CCH, N], F16)
            wv_sb = wv_pool.tile([128, CCH, DIM], F16)
            wqk_sb = wqk_pool.tile([128, CCH, 2 * DIM], F16)
            wp_sb = wp_pool.tile([128, CCH, DIM], F16)

            xT_v = xT_d[:].rearrange("(c p) n -> c p n", p=128)
            for c in range(CCH):
                nc.sync.dma_start(x_sb[:, c, :], xT_v[c])
            nc.sync.dma_start(wv_sb[:], wv_d[:].rearrange("(c p) o -> p c o", p=128))
            wqk_v = wqk_d[:].rearrange("(c p) o -> c p o", p=128)
            for c in range(CCH):
                nc.sync.dma_start(wqk_sb[:, c, :], wqk_v[c])
            nc.sync.dma_start(wp_sb[:], wp_d[:].rearrange("(c p) o -> p c o", p=128))

            ones_sb = consts.tile([1, 128], F16)
            nc.vector.memset(ones_sb[:], 1.0)
            bqk_sb = consts.tile([128, 12], F32)
            nc.sync.dma_start(bqk_sb[:], bqk_d[:])
            bv_sb = consts.tile([1, DIM], F16)
            nc.sync.dma_start(bv_sb[:], bv_d[:])
            bp_sb = consts.tile([1, DIM], F16)
            nc.sync.dma_start(bp_sb[:], bp_d[:])

            qk_sb = qk_pool.tile([128, 12, N], F16)         # [o=1536, n]
            score_sb = score_pool.tile([128, CCH, N], F16)  # [c=768, n]
            v_sb = v_pool.tile([128, 8, N_HEAD * 65], F16)  # [n, h*(64+1)]

            # ---- Phase C: v projection, natural layout + ones cols ----
            v_ones = v_sb[:].rearrange("p n (h d) -> p n h d", d=65)[:, :, :, 64:65]
            nc.vector.memset(v_ones, 1.0)
            for nt in range(8):
                for o0, ow, off in ((0, 512, 0), (512, 256, 8 * 65)):
                    ps = ps_pool.tile([128, 512], F32)
                    nc.tensor.matmul(
                        ps[:, :ow], ones_sb[:], bv_sb[:, o0:o0 + ow],
                        start=True, stop=False,
                    )
                    for c in range(CCH):
                        nc.tensor.matmul(
                            ps[:, :ow],
                            x_sb[:, c, nt * 128:(nt + 1) * 128],
                            wv_sb[:, c, o0:o0 + ow],
                            start=False,
                            stop=(c == CCH - 1),
                        )
                    nh = ow // 64
                    src = ps[:, :ow].rearrange("p (h d) -> p h d", d=64)
                    dst = v_sb[:, nt, off:off + nh * 65].rearrange(
                        "p (h d) -> p h d", d=65
                    )[:, :, 0:64]
                    nc.vector.tensor_copy(dst, src)

            # ---- Phase B helper: one [o-tile, nq] strip of the qk^T proj ----
            def b_group(ot, nq):
                ps = ps_pool.tile([128, 512], F32)
                for c in range(CCH):
                    nc.tensor.matmul(
                        ps[:],
                        wqk_sb[:, c, ot * 128:(ot + 1) * 128],
                        x_sb[:, c, nq:nq + 512],
                        start=(c == 0),
                        stop=(c == CCH - 1),
                    )
                nc.vector.tensor_scalar_add(
                    qk_sb[:, ot, nq:nq + 512], ps[:], bqk_sb[:, ot:ot + 1],
                )

            # ---- Phase D helpers ----
            def score_pair(hp, nq, nk):
                """scoresT for both heads of pair hp, one nk tile: head A into
                cols 0:512 (PE rows 0-63), head B into 512:1024 (rows 64-127),
                then exp straight from the 2-bank PSUM pair into fp16 SBUF."""
                pair = pair_pool.tile([128, 1024], F32)
                for half, p0 in ((0, 0), (1, 64)):
                    nc.tensor.matmul(
                        pair[:, half * 512:(half + 1) * 512],
                        qk_sb[p0:p0 + 64, 6 + hp, nk * 128:(nk + 1) * 128],
                        qk_sb[p0:p0 + 64, hp, nq:nq + 512],
                        start=True, stop=True,
                        tile_position=(p0, 0),
                    )
                at = attn_pool.tile([128, 1024], F16)
                nc.scalar.activation(at[:], pair[:], AF.Exp)
                return at

            def pv_group(hp, nq, half, p0, attns):
                """attn @ [v|1] for one head/nq strip + normalize by rowsum."""
                h = 2 * hp + half
                acc = acc_pool.tile([65, 512], F32)
                for nk in range(8):
                    nc.tensor.matmul(
                        acc[:],
                        v_sb[:, nk, h * 65:(h + 1) * 65],
                        attns[nk][:, half * 512:(half + 1) * 512],
                        start=(nk == 0),
                        stop=(nk == 7),
                    )
                # custom-DVE ops mis-read PSUM APs at partition offsets > 0 —
                # stage the rowsum row to SBUF first.
                rs = small_pool.tile([1, 512], F32, tag="rs")
                nc.vector.tensor_copy(rs[:], acc[64:65, :])
                rec = small_pool.tile([1, 512], F32, tag="rec")
                nc.vector.reciprocal_approx_fast(rec[:], rs[:])
                bc = small_pool.tile([64, 512], F32, tag="bc")
                nc.gpsimd.partition_broadcast(bc[:], rec[:], channels=64)
                nc.vector.tensor_mul(
                    score_sb[p0:p0 + 64, hp, nq:nq + 512], acc[0:64, :], bc[:],
                )

            # ---- Phases B + D interleaved in waves over head pairs ----
            b_group(0, 0)
            b_group(0, 512)
            b_group(6, 0)
            b_group(6, 512)
            prev_strips = None
            for hp in range(6):
                # fillers keep the PE busy while ScalarE exps this wave
                fillers = []
                if hp < 5:
                    for ot in (hp + 1, 7 + hp):
                        for nq in (0, 512):
                            fillers.append(lambda ot=ot, nq=nq: b_group(ot, nq))
                if prev_strips is not None:
                    php, pstrips = prev_strips
                    for nq in (0, 512):
                        for half, p0 in ((0, 0), (1, 64)):
                            fillers.append(
                                lambda nq=nq, half=half, p0=p0, php=php,
                                       s=pstrips: pv_group(php, nq, half, p0, s[nq])
                            )
                strips = {0: [], 512: []}
                fi = 0
                for si, (nq, nk) in enumerate(
                    [(nq, nk) for nq in (0, 512) for nk in range(8)]
                ):
                    strips[nq].append(score_pair(hp, nq, nk))
                    if si % 2 == 1 and fi < len(fillers):
                        fillers[fi]()
                        fi += 1
                while fi < len(fillers):
                    fillers[fi]()
                    fi += 1
                prev_strips = (hp, strips)

            php, pstrips = prev_strips
            for nq in (0, 512):
                for half, p0 in ((0, 0), (1, 64)):
                    pv_group(php, nq, half, p0, pstrips[nq])

            # ---- Phase E: output projection ----
            for nt in range(8):
                stage = out_pool.tile([128, DIM], F32)
                for o0, ow in ((0, 512), (512, 256)):
                    ps = ps_pool.tile([128, 512], F32)
                    nc.tensor.matmul(
                        ps[:, :ow], ones_sb[:], bp_sb[:, o0:o0 + ow],
                        start=True, stop=False,
                    )
                    for c in range(CCH):
                        nc.tensor.matmul(
                            ps[:, :ow],
                            score_sb[:, c, nt * 128:(nt + 1) * 128],
                            wp_sb[:, c, o0:o0 + ow],
                            start=False,
                            stop=(c == CCH - 1),
                        )
                    nc.vector.tensor_copy(stage[:, o0:o0 + ow], ps[:, :ow])
                nc.sync.dma_start(out_d[nt * 128:(nt + 1) * 128, :], stage[:])

    nc.compile()
    return nc


def _get_nc():
    if "nc" not in _CACHE:
        _CACHE["nc"] = _build()
    return _CACHE["nc"]


def kernel(x, w_qkv, b_qkv, w_proj, b_proj, **run_kwargs):
    x = np.asarray(x, dtype=np.float32)
    w_qkv = np.asarray(w_qkv, dtype=np.float32)
    b_qkv = np.asarray(b_qkv, dtype=np.float32)
    w_proj = np.asarray(w_proj, dtype=np.float32)
    b_proj = np.asarray(b_proj, dtype=np.float32)

    # Host-side layout prep (no arithmetic beyond folding the 1/sqrt(d) scale
    # into the q projection).
    w_qk = w_qkv[: 2 * DIM].copy()
    b_qk = b_qkv[: 2 * DIM].copy()
    w_qk[:DIM] *= SCALE
    b_qk[:DIM] *= SCALE
    w_qk_T = np.ascontiguousarray(w_qk.T).astype(np.float16)      # [768, 1536]
    b_qk_t = np.ascontiguousarray(b_qk.reshape(12, 128).T)        # [128, 12] f32
    w_v_T = np.ascontiguousarray(w_qkv[2 * DIM:].T).astype(np.float16)
    b_v = b_qkv[2 * DIM:].reshape(1, DIM).astype(np.float16)
    w_p_T = np.ascontiguousarray(w_proj.T).astype(np.float16)
    b_p = b_proj.reshape(1, DIM).astype(np.float16)

    nc = _get_nc()
    in_maps = []
    for b in range(NB):
        in_maps.append({
            "xT": np.ascontiguousarray(x[b].T).astype(np.float16),
            "w_qk_T": w_qk_T,
            "b_qk_t": b_qk_t,
            "w_v_T": w_v_T,
            "b_v": b_v,
            "w_p_T": w_p_T,
            "b_p": b_p,
        })
    res = run_bass_kernel_spmd(nc, in_maps, core_ids=list(range(NB)), **run_kwargs)
    out = np.stack([res.results[b]["out"] for b in range(NB)], axis=0)
    if run_kwargs:
        return out, res
    return out


if __name__ == "__main__":
    rng = np.random.default_rng(0)
    x = rng.standard_normal((NB, N, DIM), dtype=np.float32)
    w_qkv = rng.standard_normal((3 * DIM, DIM), dtype=np.float32) * DIM ** -0.5
    b_qkv = rng.standard_normal((3 * DIM,), dtype=np.float32) * 0.02
    w_proj = rng.standard_normal((DIM, DIM), dtype=np.float32) * DIM ** -0.5
    b_proj = rng.standard_normal((DIM,), dtype=np.float32) * 0.02
    out = kernel(x=x, w_qkv=w_qkv, b_qkv=b_qkv, w_proj=w_proj, b_proj=b_proj)
    print("out", out.shape, out.dtype, float(np.abs(out).mean()))


# revision 13
# speedup vs baseline: 1.9553x; 1.0306x over previous
"""Multi-head attention (dense transformer block) on 8 Trainium2 NeuronCores.

Reference computation (per batch element b of 8):
    qkv = x @ w_qkv.T + b_qkv                  # [1024, 2304]
    q, k, v = split heads (12 heads, d=64)
    attn = softmax(q k^T / sqrt(d))
    out  = (attn v) reshaped @ w_proj.T + b_proj

Sharding: pure data parallel — core b handles batch element b, weights are
replicated, no collectives.

Per-core kernel (all matmul operands fp16, fp32 PSUM accumulation):
  C: v    = x Wv^T + b_v            -> [1024, 12*(64+1)] (ones col per head
     makes the PV matmul emit softmax row-sums for free)
  B: qk^T = [Wq*scale; Wk] x^T      -> [1536, 1024] (features on partitions)
  D: per head pair hp: scores^T = k^T q (two heads row-tiled into the two
     halves of the PE array), exp on ScalarE straight from PSUM pairs,
     PV accumulate, normalize by approx-reciprocal(rowsum).
     B tiles for wave hp+1 and PV of wave hp-1 are interleaved between the
     score matmuls so the PE never idles while ScalarE runs the exps.
  E: out = score w_proj^T + b_proj  (bias via ones-row matmul)
"""

import os
import sys

for _p in ("/opt/trn_rl_repo", "/root/.axon_site/_ro/trn_rl_repo"):
    if os.path.isdir(_p) and _p not in sys.path:
        sys.path.insert(0, _p)

import numpy as np

import concourse.bass as bass
import concourse.mybir as mybir
import concourse.tile as tile
from concourse import bacc
from concourse.bass_utils import run_bass_kernel_spmd

DIM = 768
N_HEAD = 12
HEAD_DIM = 64
SCALE = HEAD_DIM ** (-0.5)
NB = 8          # batch == number of cores
N = 1024        # sequence length
CCH = DIM // 128  # 6 contraction chunks

F32 = mybir.dt.float32
F16 = mybir.dt.float16
AF = mybir.ActivationFunctionType

_CACHE: dict = {}


def _build():
    nc = bacc.Bacc("TRN2", target_bir_lowering=False, debug=False)

    xT_d = nc.dram_tensor("xT", [DIM, N], F16, kind="ExternalInput")
    wqk_d = nc.dram_tensor("w_qk_T", [DIM, 2 * DIM], F16, kind="ExternalInput")
    bqk_d = nc.dram_tensor("b_qk_t", [128, 12], F32, kind="ExternalInput")
    wv_d = nc.dram_tensor("w_v_T", [DIM, DIM], F16, kind="ExternalInput")
    bv_d = nc.dram_tensor("b_v", [1, DIM], F16, kind="ExternalInput")
    wp_d = nc.dram_tensor("w_p_T", [DIM, DIM], F16, kind="ExternalInput")
    bp_d = nc.dram_tensor("b_p", [1, DIM], F16, kind="ExternalInput")
    out_d = nc.dram_tensor("out", [N, DIM], F32, kind="ExternalOutput")

    with tile.TileContext(nc) as tc:
        with (
            tc.tile_pool(name="consts", bufs=1) as consts,
            tc.tile_pool(name="qk", bufs=1) as qk_pool,
            tc.tile_pool(name="score", bufs=1) as score_pool,
            tc.tile_pool(name="v", bufs=1) as v_pool,
            tc.tile_pool(name="x", bufs=1) as x_pool,
            tc.tile_pool(name="wqk", bufs=1) as wqk_pool,
            tc.tile_pool(name="wv", bufs=1) as wv_pool,
            tc.tile_pool(name="wp", bufs=1) as wp_pool,
            tc.tile_pool(name="attn", bufs=32) as attn_pool,
            tc.tile_pool(name="small", bufs=4) as small_pool,
            tc.tile_pool(name="ostage", bufs=3) as out_pool,
            tc.tile_pool(name="ps", bufs=2, space="PSUM") as ps_pool,
            tc.tile_pool(name="pair", bufs=2, space="PSUM") as pair_pool,
            tc.tile_pool(name="acc", bufs=2, space="PSUM") as acc_pool,
        ):
            x_sb = x_pool.tile([128, CCH, N], F16)
            wv_sb = wv_pool.tile([128, CCH, DIM], F16)
            wqk_sb = wqk_pool.tile([128, CCH, 2 * DIM], F16)
            wp_sb = wp_pool.tile([128, CCH, DIM], F16)

            ones_sb = consts.tile([1, 128], F16)
            nc.vector.memset(ones_sb[:], 1.0)
            bqk_sb = consts.tile([128, 12], F32)
            nc.sync.dma_start(bqk_sb[:], bqk_d[:])
            bv_sb = consts.tile([1, DIM], F16)
            nc.sync.dma_start(bv_sb[:], bv_d[:])
            bp_sb = consts.tile([1, DIM], F16)
            nc.sync.dma_start(bp_sb[:], bp_d[:])

            xT_v = xT_d[:].rearrange("(c p) n -> c p n", p=128)
            for c in range(CCH):
                nc.sync.dma_start(x_sb[:, c, :], xT_v[c])
            nc.sync.dma_start(wv_sb[:], wv_d[:].rearrange("(c p) o -> p c o", p=128))
            wqk_v = wqk_d[:].rearrange("(c p) o -> c p o", p=128)
            for c in range(CCH):
                nc.sync.dma_start(wqk_sb[:, c, :], wqk_v[c])
            nc.sync.dma_start(wp_sb[:], wp_d[:].rearrange("(c p) o -> p c o", p=128))

            qk_sb = qk_pool.tile([128, 12, N], F16)         # [o=1536, n]
            score_sb = score_pool.tile([128, CCH, N], F16)  # [c=768, n]
            v_sb = v_pool.tile([128, 8, N_HEAD * 65], F16)  # [n, h*(64+1)]

            # ---- Phase C: v projection, natural layout + ones cols ----
            v_ones = v_sb[:].rearrange("p n (h d) -> p n h d", d=65)[:, :, :, 64:65]
            nc.vector.memset(v_ones, 1.0)
            for nt in range(8):
                for o0, ow, off in ((0, 512, 0), (512, 256, 8 * 65)):
                    ps = ps_pool.tile([128, 512], F32)
                    nc.tensor.matmul(
                        ps[:, :ow], ones_sb[:], bv_sb[:, o0:o0 + ow],
                        start=True, stop=False,
                    )
                    for c in range(CCH):
                        nc.tensor.matmul(
                            ps[:, :ow],
                            x_sb[:, c, nt * 128:(nt + 1) * 128],
                            wv_sb[:, c, o0:o0 + ow],
                            start=False,
                            stop=(c == CCH - 1),
                        )
                    nh = ow // 64
                    src = ps[:, :ow].rearrange("p (h d) -> p h d", d=64)
                    dst = v_sb[:, nt, off:off + nh * 65].rearrange(
                        "p (h d) -> p h d", d=65
                    )[:, :, 0:64]
                    nc.vector.tensor_copy(dst, src)

            # ---- Phase B helper: one [o-tile, nq] strip of the qk^T proj ----
            def b_group(ot, nq):
                ps = ps_pool.tile([128, 512], F32)
                for c in range(CCH):
                    nc.tensor.matmul(
                        ps[:],
                        wqk_sb[:, c, ot * 128:(ot + 1) * 128],
                        x_sb[:, c, nq:nq + 512],
                        start=(c == 0),
                        stop=(c == CCH - 1),
                    )
                nc.vector.tensor_scalar_add(
                    qk_sb[:, ot, nq:nq + 512], ps[:], bqk_sb[:, ot:ot + 1],
                )

            # ---- Phase D helpers ----
            def score_pair(hp, nq, nk):
                """scoresT for both heads of pair hp, one nk tile: head A into
                cols 0:512 (PE rows 0-63), head B into 512:1024 (rows 64-127),
                then exp straight from the 2-bank PSUM pair into fp16 SBUF."""
                pair = pair_pool.tile([128, 1024], F32)
                for half, p0 in ((0, 0), (1, 64)):
                    nc.tensor.matmul(
                        pair[:, half * 512:(half + 1) * 512],
                        qk_sb[p0:p0 + 64, 6 + hp, nk * 128:(nk + 1) * 128],
                        qk_sb[p0:p0 + 64, hp, nq:nq + 512],
                        start=True, stop=True,
                        tile_position=(p0, 0),
                    )
                at = attn_pool.tile([128, 1024], F16)
                nc.scalar.activation(at[:], pair[:], AF.Exp)
                return at

            def pv_group(hp, nq, half, p0, attns):
                """attn @ [v|1] for one head/nq strip + normalize by rowsum."""
                h = 2 * hp + half
                acc = acc_pool.tile([65, 512], F32)
                for nk in range(8):
                    nc.tensor.matmul(
                        acc[:],
                        v_sb[:, nk, h * 65:(h + 1) * 65],
                        attns[nk][:, half * 512:(half + 1) * 512],
                        start=(nk == 0),
                        stop=(nk == 7),
                    )
                # custom-DVE ops mis-read PSUM APs at partition offsets > 0 —
                # stage the rowsum row to SBUF first.
                rs = small_pool.tile([1, 512], F32, tag="rs")
                nc.vector.tensor_copy(rs[:], acc[64:65, :])
                rec = small_pool.tile([1, 512], F32, tag="rec")
                nc.vector.reciprocal_approx_fast(rec[:], rs[:])
                bc = small_pool.tile([64, 512], F32, tag="bc")
                nc.gpsimd.partition_broadcast(bc[:], rec[:], channels=64)
                nc.vector.tensor_mul(
                    score_sb[p0:p0 + 64, hp, nq:nq + 512], acc[0:64, :], bc[:],
                )

            # ---- Phases B + D interleaved in waves over head pairs ----
            b_group(0, 0)
            b_group(0, 512)
            b_group(6, 0)
            b_group(6, 512)
            prev_strips = None
            for hp in range(6):
                # fillers keep the PE busy while ScalarE exps this wave
                fillers = []
                if hp < 5:
                    for ot in (hp + 1, 7 + hp):
                        for nq in (0, 512):
                            fillers.append(lambda ot=ot, nq=nq: b_group(ot, nq))
                if prev_strips is not None:
                    php, pstrips = prev_strips
                    for nq in (0, 512):
                        for half, p0 in ((0, 0), (1, 64)):
                            fillers.append(
                                lambda nq=nq, half=half, p0=p0, php=php,
                                       s=pstrips: pv_group(php, nq, half, p0, s[nq])
                            )
                strips = {0: [], 512: []}
                fi = 0
                for si, (nq, nk) in enumerate(
                    [(nq, nk) for nq in (0, 512) for nk in range(8)]
                ):
                    strips[nq].append(score_pair(hp, nq, nk))
                    if si % 2 == 1 and fi < len(fillers):
                        fillers[fi]()
                        fi += 1
                while fi < len(fillers):
                    fillers[fi]()
                    fi += 1
                prev_strips = (hp, strips)

            # ---- Phase E helper ----
            def e_tile(nt):
                stage = out_pool.tile([128, DIM], F32)
                for o0, ow in ((0, 512), (512, 256)):
                    ps = ps_pool.tile([128, 512], F32)
                    nc.tensor.matmul(
                        ps[:, :ow], ones_sb[:], bp_sb[:, o0:o0 + ow],
                        start=True, stop=False,
                    )
                    for c in range(CCH):
                        nc.tensor.matmul(
                            ps[:, :ow],
                            score_sb[:, c, nt * 128:(nt + 1) * 128],
                            wp_sb[:, c, o0:o0 + ow],
                            start=False,
                            stop=(c == CCH - 1),
                        )
                    nc.vector.tensor_copy(stage[:, o0:o0 + ow], ps[:, :ow])
                nc.sync.dma_start(out_d[nt * 128:(nt + 1) * 128, :], stage[:])

            # final pv wave, with E tiles slotted in once their nq half of
            # score_sb is fully normalized
            php, pstrips = prev_strips
            for nq, nts in ((0, (0, 1, 2, 3)), (512, (4, 5, 6, 7))):
                for half, p0 in ((0, 0), (1, 64)):
                    pv_group(php, nq, half, p0, pstrips[nq])
                for nt in nts:
                    e_tile(nt)

    nc.compile()
    return nc


def _get_nc():
    if "nc" not in _CACHE:
        _CACHE["nc"] = _build()
    return _CACHE["nc"]


def kernel(x, w_qkv, b_qkv, w_proj, b_proj, **run_kwargs):
    x = np.asarray(x, dtype=np.float32)
    w_qkv = np.asarray(w_qkv, dtype=np.float32)
    b_qkv = np.asarray(b_qkv, dtype=np.float32)
    w_proj = np.asarray(w_proj, dtype=np.float32)
    b_proj = np.asarray(b_proj, dtype=np.float32)

    # Host-side layout prep (no arithmetic beyond folding the 1/sqrt(d) scale
    # into the q projection).
    w_qk = w_qkv[: 2 * DIM].copy()
    b_qk = b_qkv[: 2 * DIM].copy()
    w_qk[:DIM] *= SCALE
    b_qk[:DIM] *= SCALE
    w_qk_T = np.ascontiguousarray(w_qk.T).astype(np.float16)      # [768, 1536]
    b_qk_t = np.ascontiguousarray(b_qk.reshape(12, 128).T)        # [128, 12] f32
    w_v_T = np.ascontiguousarray(w_qkv[2 * DIM:].T).astype(np.float16)
    b_v = b_qkv[2 * DIM:].reshape(1, DIM).astype(np.float16)
    w_p_T = np.ascontiguousarray(w_proj.T).astype(np.float16)
    b_p = b_proj.reshape(1, DIM).astype(np.float16)

    nc = _get_nc()
    in_maps = []
    for b in range(NB):
        in_maps.append({
            "xT": np.ascontiguousarray(x[b].T).astype(np.float16),
            "w_qk_T": w_qk_T,
            "b_qk_t": b_qk_t,
            "w_v_T": w_v_T,
            "b_v": b_v,
            "w_p_T": w_p_T,
            "b_p": b_p,
        })
    res = run_bass_kernel_spmd(nc, in_maps, core_ids=list(range(NB)), **run_kwargs)
    out = np.stack([res.results[b]["out"] for b in range(NB)], axis=0)
    if run_kwargs:
        return out, res
    return out


if __name__ == "__main__":
    rng = np.random.default_rng(0)
    x = rng.standard_normal((NB, N, DIM), dtype=np.float32)
    w_qkv = rng.standard_normal((3 * DIM, DIM), dtype=np.float32) * DIM ** -0.5
    b_qkv = rng.standard_normal((3 * DIM,), dtype=np.float32) * 0.02
    w_proj = rng.standard_normal((DIM, DIM), dtype=np.float32) * DIM ** -0.5
    b_proj = rng.standard_normal((DIM,), dtype=np.float32) * 0.02
    out = kernel(x=x, w_qkv=w_qkv, b_qkv=b_qkv, w_proj=w_proj, b_proj=b_proj)
    print("out", out.shape, out.dtype, float(np.abs(out).mean()))
